# revision 19
# baseline (speedup 1.0000x reference)
"""AttractorPooling (correlation-dimension) kernel for 8 Trainium2 NeuronCores.

Batch b -> core b (data parallel, SPMD: one program, per-core inputs). Each
core computes its batch's pairwise squared distances bit-exactly the way the
jax reference does:
    G  = x @ x.T          (K=3 f32 matmul on the PE; bit-matches XLA's einsum)
    a  = fl(sq_i + sq_j)  (one f32 add on the DVE; sq from host, left-to-right)
    d2 = fl(-2*G + a)     (one f32 scalar_tensor_tensor rounding on the DVE)

Tiling exploits symmetry: the strict block-upper triangle is counted once and
weighted x2, the 32 diagonal 128x128 blocks are counted once with the i==j
entries pushed out of range by a +1000*I mask folded into `a`. Because the
PE rounds stationary/moving operands asymmetrically, the oracle's d2 is NOT
bit-symmetric, so the sensitive smallest-radius count is additionally
computed over the mirror (block-lower) strips and decoded without the x2.

Counting d < r is reduced to d2 < T(r), where T(r) is the smallest f32 whose
correctly-rounded sqrt is >= r (host-derived), making the exact-f32 t=0
count equivalent to the reference's sqrt/clip comparison. The other 19
thresholds are counted on a bf16 copy of d2 against thresholds nudged
strictly between bf16 grid points (tie-free); for log-spaced radii their
counts influence the final mean-of-slopes output only at the ~1e-7 level
(the telescoping mean depends on the end-point counts).

Counting passes are fused compare+accumulate instructions balanced across
the ACT engine (Sign activation, accumulation is free there: t=0 + 11
thresholds) and the DVE (tensor_scalar is_lt + accum_out, 8 thresholds);
measured on HW, all accumulating passes run at 1 element/lane/cycle, which
is what sets the kernel's ~1.2 ms/core runtime. Per-partition counts land in
per-(tile, threshold) strip columns, are reduced across partitions with a
ones-matmul on the otherwise-idle PE, and the [B,20] -> [B] log-slope
finish happens on the host.
"""

import sys

if "/opt/trn_rl_repo" not in sys.path:
    sys.path.insert(0, "/opt/trn_rl_repo")

from contextlib import ExitStack

import ml_dtypes
import numpy as np

import concourse.bacc as bacc
import concourse.tile as tile
from concourse import mybir
from concourse.alu_op_type import AluOpType
from concourse.bass_utils import run_bass_kernel_spmd

B, N, D = 8, 4096, 3
P = 128  # partition block
FMAX = 2048  # macro tile width (cols)
MMF = 512  # matmul moving free-dim chunk (one PSUM bank of f32)
R = 20  # number of radii
EPS = 1e-8

# Engine assignment: t=0 runs in exact f32 as an ACT Sign pass; the bf16
# thresholds are split across ACT (Sign) and DVE (is_lt) to balance the two
# 1x-rate counting engines. (GPSIMD cannot run accumulating tensor_scalar:
# the opcode fails the NEURON_ISA engine check at codegen.)
ACT_T = list(range(1, 12))
GPS_T = []
DVE_BF_T = [t for t in range(1, R) if t not in ACT_T and t not in GPS_T]


def _plan_tiles():
    """Macro tiles: ('u', row_block, col0, width) or ('d', first_row_block, 0, w).

    Upper tiles cover cols [128*(r+1), 4096) of row-block r (strict upper
    triangle, weight 2). Diag tiles pack 16 diagonal 128x128 blocks side by
    side (weight 1)."""
    tiles = []
    for r in range(N // P - 1):
        c0 = P * (r + 1)
        w_total = N - c0
        off = 0
        while off < w_total:
            w = min(FMAX, w_total - off)
            tiles.append(("u", r, c0 + off, w))
            off += w
    n_diag_macro = (N // P) // (FMAX // P)
    for dblk in range(n_diag_macro):
        tiles.append(("d", dblk * (FMAX // P), 0, FMAX))
    # mirror (lower-triangle) strips: cols [0, 128*r) of row-block r.
    # The oracle's d2 is not bit-symmetric (PE stationary/moving roles round
    # differently), so the sensitive t=0 count is computed on both triangles
    # exactly; these tiles only run the f32 t=0 compare.
    for r in range(1, N // P):
        w_total = P * r
        off = 0
        while off < w_total:
            w = min(FMAX, w_total - off)
            tiles.append(("l", r, off, w))
            off += w
    return tiles


TILES = _plan_tiles()
NT = len(TILES)
ND = 1 + len(DVE_BF_T) + len(GPS_T)  # strip cols per full tile (t0+DVE+GPS)
NA = len(ACT_T)

# Strip column layout: upper/diag tiles get ND DVE cols + NA ACT cols;
# mirror tiles get a single DVE col (t=0 only).
TILE_DVE_OFF = []
TILE_ACT_OFF = []
_d_off = 0
_a_off = 0
for _kind, _r0, _c0, _w in TILES:
    TILE_DVE_OFF.append(_d_off)
    TILE_ACT_OFF.append(_a_off)
    if _kind in ("u", "d"):
        _d_off += ND
        _a_off += NA
    else:
        _d_off += 1
N_DVE_COLS = _d_off
N_ACT_COLS = _a_off


def _sqrt_boundary(radii_f32: np.ndarray) -> np.ndarray:
    """T(r): smallest f32 x >= 0 with f32-sqrt(x) >= r. Then
    (sqrt(clip(d2, EPS)) < r) == (d2 < T(r)) for all f32 d2 (EPS < T always
    holds here since r >= 1e-3 -> T >= 1e-6 > 1e-8)."""
    out = np.empty(R, np.float32)
    for i, r in enumerate(radii_f32):
        x = np.float32(r) * np.float32(r)
        # walk down while sqrt still >= r, then up while sqrt < r
        while x > 0 and np.sqrt(np.float32(np.nextafter(x, np.float32(0.0), dtype=np.float32))) >= r:
            x = np.nextafter(x, np.float32(0.0), dtype=np.float32)
        while np.sqrt(x) < r:
            x = np.nextafter(x, np.float32(np.inf), dtype=np.float32)
        # reference compares sqrt(max(d2, EPS)) < r: if T <= EPS nothing
        # can ever be below r (d2 is always > -1), encode as threshold -1
        out[i] = x if x > np.float32(EPS) else np.float32(-1.0)
    return out


def _nudge_bf16(ts: np.ndarray) -> np.ndarray:
    """For each f32 threshold T>0 return T' strictly between the bf16 grid
    points bracketing T, such that (bf16 v) < T'  <=>  v < T, no v == T'."""
    out = np.empty_like(ts, dtype=np.float64)
    for i, t in enumerate(ts.astype(np.float64)):
        v = np.float32(t).astype(ml_dtypes.bfloat16)
        bits = v.view(np.uint16)
        vf = np.float64(v.astype(np.float32))
        if vf >= t:
            hi = vf
            lo = np.float64((bits - 1).astype(np.uint16).view(ml_dtypes.bfloat16).astype(np.float32))
        else:
            lo = vf
            hi = np.float64((bits + 1).astype(np.uint16).view(ml_dtypes.bfloat16).astype(np.float32))
        out[i] = 0.5 * (lo + hi)
    return out.astype(np.float32)


def _build_program(thr_f32: np.ndarray, thr_bf: np.ndarray, n_reps: int = 1):
    """thr_f32: exact f32 boundaries T(r_t); thr_bf: bf16-nudged versions.

    n_reps > 1 wraps the compute body in an on-device loop (identical,
    idempotent iterations) -- used only for timing measurements."""
    nc = bacc.Bacc(
        "TRN2",
        target_bir_lowering=False,
        debug=False,
        enable_asserts=False,
        num_devices=B,
    )
    f32 = mybir.dt.float32
    bf16 = mybir.dt.bfloat16

    xT_d = nc.dram_tensor("xT", [3, N], f32, kind="ExternalInput").ap()
    sqj_d = nc.dram_tensor("sqj", [1, N], f32, kind="ExternalInput").ap()
    sqi_d = nc.dram_tensor("sqi", [P, N // P], f32, kind="ExternalInput").ap()
    negth_d = nc.dram_tensor("negth", [P, R], f32, kind="ExternalInput").ap()
    mask_d = nc.dram_tensor("mask128", [P, P], f32, kind="ExternalInput").ap()

    accd_out = nc.dram_tensor("acc_dve", [1, N_DVE_COLS], f32, kind="ExternalOutput").ap()
    acca_out = nc.dram_tensor("acc_act", [1, N_ACT_COLS], f32, kind="ExternalOutput").ap()

    with tile.TileContext(nc) as tc:
        with ExitStack() as ctx:
            cpool = ctx.enter_context(tc.tile_pool(name="const", bufs=1))
            xt = cpool.tile([3, N], f32, tag="xt")
            sqj = cpool.tile([P, N], f32, tag="sqj")
            sqi = cpool.tile([P, N // P], f32, tag="sqi")
            negtht = cpool.tile([P, R], f32, tag="negth")
            maskt = cpool.tile([P, P], f32, tag="mask")
            onest = cpool.tile([P, 1], f32, tag="ones")
            accs_d = cpool.tile([P, N_DVE_COLS], f32, tag="accd")
            accs_a = cpool.tile([P, N_ACT_COLS], f32, tag="acca")

            nc.sync.dma_start(xt[:], xT_d[:])
            nc.sync.dma_start(sqi[:], sqi_d[:])
            nc.sync.dma_start(negtht[:], negth_d[:])
            nc.sync.dma_start(maskt[:], mask_d[:])
            # replicate sq across all 128 partitions with broadcast-read DMAs,
            # split so early tiles start sooner
            for c in range(4):
                cs = N // 4
                src = sqj_d[0:1, c * cs : (c + 1) * cs].broadcast_to((P, cs))
                nc.sync.dma_start(sqj[:, c * cs : (c + 1) * cs], src)
            nc.vector.memset(onest[:], 1.0)

            with ExitStack() as ctx2:
                pspool = ctx2.enter_context(
                    tc.tile_pool(name="ps", bufs=2, space="PSUM")
                )
                apool = ctx2.enter_context(tc.tile_pool(name="apool", bufs=3))
                d2pool = ctx2.enter_context(tc.tile_pool(name="d2pool", bufs=3))
                convp = ctx2.enter_context(tc.tile_pool(name="conv", bufs=4))
                scrdp = ctx2.enter_context(tc.tile_pool(name="scrd", bufs=2))
                scrap = ctx2.enter_context(tc.tile_pool(name="scra", bufs=2))
                if n_reps > 1:
                    rep_loop = ctx2.enter_context(tc.For_i(0, n_reps, 1))

                for m, (kind, r0, c0, w) in enumerate(TILES):
                    do = TILE_DVE_OFF[m]
                    ao = TILE_ACT_OFF[m]
                    ps = pspool.tile([P, FMAX], f32, tag="ps")
                    if kind in ("u", "l"):
                        lhsT = xt[:, P * r0 : P * (r0 + 1)]
                        off = 0
                        while off < w:
                            ww = min(MMF, w - off)
                            nc.tensor.matmul(
                                ps[:, off : off + ww],
                                lhsT,
                                xt[:, c0 + off : c0 + off + ww],
                                start=True,
                                stop=True,
                            )
                            off += ww
                        # a = fl(sq_i + sq_j) on this tile's column range
                        asb = apool.tile([P, FMAX], f32, tag="asb")
                        nc.vector.tensor_scalar(
                            asb[:, :w],
                            sqj[:, c0 : c0 + w],
                            sqi[:, r0 : r0 + 1],
                            None,
                            AluOpType.add,
                        )
                    else:
                        # 16 diagonal 128x128 blocks side by side
                        for q in range(FMAX // P):
                            rr = r0 + q
                            nc.tensor.matmul(
                                ps[:, P * q : P * (q + 1)],
                                xt[:, P * rr : P * (rr + 1)],
                                xt[:, P * rr : P * (rr + 1)],
                                start=True,
                                stop=True,
                            )
                        asb0 = apool.tile([P, FMAX], f32, tag="asb0")
                        for q in range(FMAX // P):
                            rr = r0 + q
                            nc.vector.tensor_scalar(
                                asb0[:, P * q : P * (q + 1)],
                                sqj[:, P * rr : P * (rr + 1)],
                                sqi[:, rr : rr + 1],
                                None,
                                AluOpType.add,
                            )
                        # push the i==j entries out of every threshold's range:
                        # a += 1000*I (off-diagonal entries add exact 0)
                        asb = apool.tile([P, FMAX], f32, tag="asb")
                        mask_rep = maskt[:, :].unsqueeze(1).broadcast_to(
                            (P, FMAX // P, P)
                        )
                        nc.vector.tensor_tensor(
                            asb[:, :w], asb0[:, :w], mask_rep, AluOpType.add
                        )

                    # d2 = fl(-2*G + a)  (bit-exact vs reference)
                    d2sb = d2pool.tile([P, FMAX], f32, tag="d2sb")
                    nc.vector.scalar_tensor_tensor(
                        d2sb[:, :w],
                        ps[:, :w],
                        -2.0,
                        asb[:, :w],
                        AluOpType.mult,
                        AluOpType.add,
                    )
                    scrd = scrdp.tile([P, FMAX], bf16, tag="scrd")
                    # t=0 exact on f32 d2 via ACT Sign (accum is free on ACT)
                    nc.scalar.activation(
                        scrd[:, :w],
                        d2sb[:, :w],
                        mybir.ActivationFunctionType.Sign,
                        bias=negtht[:, 0:1],
                        scale=1.0,
                        accum_out=accs_d[:, do : do + 1],
                    )
                    if kind == "l":
                        continue

                    # bf16 copy for the fast threshold passes
                    conv = convp.tile([P, FMAX], bf16, tag="conv")
                    nc.vector.tensor_copy(conv[:, :w], d2sb[:, :w])
                    for j, t in enumerate(DVE_BF_T):
                        nc.vector.tensor_scalar(
                            scrd[:, :w],
                            conv[:, :w],
                            float(thr_bf[t]),
                            0.0,
                            AluOpType.is_lt,
                            AluOpType.add,
                            accum_out=accs_d[:, do + 1 + j : do + 2 + j],
                        )
                    scra = scrap.tile([P, FMAX], bf16, tag="scra")
                    for j, t in enumerate(ACT_T):
                        nc.scalar.activation(
                            scra[:, :w],
                            conv[:, :w],
                            mybir.ActivationFunctionType.Sign,
                            bias=negtht[:, t : t + 1],
                            scale=1.0,
                            accum_out=accs_a[:, ao + j : ao + j + 1],
                        )

            # Reduce partition dim with ones-matmuls on PE, then DMA out.
            with ExitStack() as ctx3:
                redp = ctx3.enter_context(
                    tc.tile_pool(name="red", bufs=2, space="PSUM")
                )
                outp = ctx3.enter_context(tc.tile_pool(name="outp", bufs=1))
                osb_d = outp.tile([1, N_DVE_COLS], f32, tag="osbd")
                osb_a = outp.tile([1, N_ACT_COLS], f32, tag="osba")
                for accs, total, osb, dram in (
                    (accs_d, N_DVE_COLS, osb_d, accd_out),
                    (accs_a, N_ACT_COLS, osb_a, acca_out),
                ):
                    off = 0
                    while off < total:
                        ww = min(MMF, total - off)
                        rp = redp.tile([1, MMF], f32, tag="red")
                        nc.tensor.matmul(
                            rp[0:1, :ww],
                            onest[:],
                            accs[:, off : off + ww],
                            start=True,
                            stop=True,
                        )
                        nc.vector.tensor_copy(osb[0:1, off : off + ww], rp[0:1, :ww])
                        off += ww
                    nc.sync.dma_start(dram[:], osb[:])

    nc.compile()
    return nc


_PROGRAM_CACHE: dict = {}


def _get_program(thr_f32: np.ndarray, thr_bf: np.ndarray):
    key = (thr_f32.tobytes(), thr_bf.tobytes())
    if key not in _PROGRAM_CACHE:
        _PROGRAM_CACHE[key] = _build_program(thr_f32, thr_bf)
    return _PROGRAM_CACHE[key]


def _host_inputs(trajectory: np.ndarray, thr_bf: np.ndarray, thr_f32: np.ndarray = None):
    """Per-core in_maps. sq computed left-to-right in f32 exactly as the
    reference's jnp.sum(x*x, axis=2)."""
    x = trajectory.astype(np.float32)
    sq = (x[:, :, 0] * x[:, :, 0] + x[:, :, 1] * x[:, :, 1]) + x[:, :, 2] * x[:, :, 2]
    sq = sq.astype(np.float32)  # [B,N]
    negth = np.tile(-thr_bf[None, :], (P, 1)).astype(np.float32)  # [128, R]
    if thr_f32 is not None:
        negth[:, 0] = -thr_f32[0]  # t=0 uses the exact f32 boundary (Sign pass)
    mask128 = (np.eye(P, dtype=np.float32) * 1000.0).astype(np.float32)
    in_maps = []
    for b in range(B):
        in_maps.append(
            {
                "xT": np.ascontiguousarray(x[b].T),
                "sqj": np.ascontiguousarray(sq[b][None, :]),
                "sqi": np.ascontiguousarray(sq[b].reshape(N // P, P).T),
                "negth": negth,
                "mask128": mask128,
            }
        )
    return in_maps


def _decode_counts(acc_dve: np.ndarray, acc_act: np.ndarray) -> np.ndarray:
    """[1, N_DVE_COLS], [1, N_ACT_COLS] -> counts[R] over ordered pairs i != j.

    t=0 is summed over upper + diag + mirror tiles (weight 1 each, covering
    the full off-diagonal matrix exactly); other thresholds use the
    symmetrized upper*2 + diag counts."""
    ad = acc_dve.ravel().astype(np.float64)
    aa = acc_act.ravel().astype(np.float64)
    counts = np.zeros(R, np.float64)
    for m, (kind, r0, c0, w) in enumerate(TILES):
        do = TILE_DVE_OFF[m]
        ao = TILE_ACT_OFF[m]
        counts[0] += (P * w - ad[do]) / 2.0
        if kind == "l":
            continue
        wgt = 2.0 if kind == "u" else 1.0
        n_m = P * w
        for j, t in enumerate(DVE_BF_T):
            counts[t] += wgt * ad[do + 1 + j]
        for j, t in enumerate(GPS_T):
            counts[t] += wgt * ad[do + 1 + len(DVE_BF_T) + j]
        for j, t in enumerate(ACT_T):
            counts[t] += wgt * (n_m - aa[ao + j]) / 2.0
    return counts


def _slope_from_counts(counts: np.ndarray, radii: np.ndarray) -> np.float64:
    total_pairs = float(N * (N - 1))
    log_c = np.log(counts / total_pairs + EPS)
    log_r = np.log(radii.astype(np.float64) + EPS)
    slopes = (log_c[1:] - log_c[:-1]) / (log_r[1:] - log_r[:-1])
    return np.clip(np.mean(slopes), 0.1, 3.0)


def _thresholds(radii: np.ndarray):
    radii_f32 = radii.astype(np.float32)
    thr_f32 = _sqrt_boundary(radii_f32)
    thr_bf = _nudge_bf16(thr_f32)
    return thr_f32, thr_bf


def kernel(trajectory: np.ndarray, radii: np.ndarray) -> np.ndarray:
    assert trajectory.shape == (B, N, D), trajectory.shape
    assert radii.shape == (R,), radii.shape
    radii_f32 = radii.astype(np.float32)
    thr_f32, thr_bf = _thresholds(radii_f32)

    nc = _get_program(thr_f32, thr_bf)
    in_maps = _host_inputs(trajectory, thr_bf, thr_f32)
    res = run_bass_kernel_spmd(nc, in_maps, core_ids=list(range(B)))

    out = np.empty(B, np.float32)
    for b in range(B):
        counts = _decode_counts(res.results[b]["acc_dve"], res.results[b]["acc_act"])
        out[b] = np.float32(_slope_from_counts(counts, radii_f32))
    return out


if __name__ == "__main__":
    rng = np.random.default_rng(0)
    traj = rng.standard_normal((B, N, D), dtype=np.float32)
    radii = np.logspace(np.log10(1e-3), np.log10(10.0), R).astype(np.float32)
    print(kernel(traj, radii))
